# revision 12
# baseline (speedup 1.0000x reference)
"""Trainium2 bass kernel for the GNN message-passing problem.

kernel(**inputs) -> np.ndarray [100000, 1]

Strategy: edges sharded contiguously across 8 NeuronCores (200k/core).
Host packs per-edge features as x = [r, vx, vy, vz] in a [4, K]
feature-major layout (r = |r_ij|/H precomputed on host), so the device
needs no transposes: each teacher layer is a plain wide matmul.

Per 1024-edge chunk, per direction (dir j uses a stationary with the
v-columns negated, reusing the same rhs):
  z0 = W0 @ x      (2x 512-col float32r matmuls -> PSUM [128,1024])
  h0 = silu(z0+b0) (1 activation -> SBUF)
  z1 = W1 @ h0     (2x 512-col matmuls)
  h1 = silu(z1+b1)
  y  = w2 . h1     (1x 512-col matmul per 512-edge sub-block; the
                    stationary w2emb[t] = w2 embedded in column t of a
                    [128,128] zero matrix deposits the result into
                    partition t of an accumulating [128,512] PSUM bank
                    - fp32r matmul PSUM writes must start at partition
                    0, so rows are accumulated in, not addressed.
                    Dense [128,512] y blocks get one DVE copy + DMA
                    per 128 sub-block-dirs.)
float32r keeps fp32 precision at 1 cycle/row PE throughput.
b2 and the index-dependent segment-sum/count/divide run on the host.
"""
import sys
sys.path.insert(0, "/opt/trn_rl_repo")
import numpy as np

N_NODES = 100000
E_TOTAL = 1600000
HSM = 3.0
N_CORES = 8
CHUNK = 1024            # edges per chunk (2 PSUM banks wide)
SB = CHUNK // 512       # 512-col sub-blocks per chunk
Epc = E_TOTAL // N_CORES
NCH = (Epc + CHUNK - 1) // CHUNK   # 196
K = NCH * CHUNK                    # 200704 padded edges per core
XS = 4096               # x super-tile columns (4 chunks per DMA)
SPC = NCH * 2 * SB      # sub-block-dirs per core: 784


def _split_multi_waits(nc, max_waits=1):
    """This walrus build rejects >1 sync-wait on CTRL ops (Tile's final
    drain). Move extra waits onto preceding single-wait InstNoOps."""
    import concourse.mybir as mybir
    n_split = 0
    for f in nc.m.functions:
        for blk in f.blocks:
            insns = blk.instructions
            out = []
            for ins in insns:
                si = ins.sync_info
                if si is not None and si.on_wait and len(si.on_wait) > max_waits:
                    waits = list(si.on_wait)
                    for k, w in enumerate(waits[:-max_waits]):
                        nop = mybir.InstNoOp(name=f"{ins.name}-ws{k}")
                        nop.engine = ins.engine
                        nop.sync_info = mybir.SyncInfo(on_wait=[w], on_update=[])
                        out.append(nop)
                        n_split += 1
                    si.on_wait = waits[-max_waits:]
                out.append(ins)
            blk.instructions = out
    return n_split


def _build_kernel():
    import concourse.bass as bass
    import concourse.mybir as mybir
    from concourse.tile import TileContext

    F32 = mybir.dt.float32
    F32R = mybir.dt.float32r
    AF = mybir.ActivationFunctionType

    nc = bass.Bass()
    x_d = nc.declare_dram_parameter("x", [4, K], F32R, isOutput=False)
    w0p_d = nc.declare_dram_parameter("w0p", [4, 128], F32R, isOutput=False)
    w0n_d = nc.declare_dram_parameter("w0n", [4, 128], F32R, isOutput=False)
    w1_d = nc.declare_dram_parameter("w1lt", [128, 128], F32R, isOutput=False)
    w2e_d = nc.declare_dram_parameter("w2e", [128, 128 * 128], F32R,
                                      isOutput=False)
    b0_d = nc.declare_dram_parameter("b0c", [128, 1], F32, isOutput=False)
    b1_d = nc.declare_dram_parameter("b1c", [128, 1], F32, isOutput=False)
    y_d = nc.declare_dram_parameter("y", [SPC, 512], F32, isOutput=True)

    with TileContext(nc) as tc:
        with (
            tc.tile_pool(name="const", bufs=1) as cpool,
            tc.tile_pool(name="xp", bufs=2) as xp,
            tc.tile_pool(name="hp", bufs=4) as hp,
            tc.tile_pool(name="zp", bufs=3, space="PSUM") as zp,
            tc.tile_pool(name="yp", bufs=2, space="PSUM") as yp,
            tc.tile_pool(name="ysp", bufs=2) as ysp,
        ):
            w0pt = cpool.tile([4, 128], F32R, tag="w0p")
            w0nt = cpool.tile([4, 128], F32R, tag="w0n")
            w1t = cpool.tile([128, 128], F32R, tag="w1")
            w2et = cpool.tile([128, 128 * 128], F32R, tag="w2e")
            b0t = cpool.tile([128, 1], F32, tag="b0")
            b1t = cpool.tile([128, 1], F32, tag="b1")
            for tgt, src in ((w0pt, w0p_d), (w0nt, w0n_d), (w1t, w1_d),
                             (w2et, w2e_d), (b0t, b0_d), (b1t, b1_d)):
                nc.sync.dma_start(out=tgt[:], in_=src[:])

            xt = None
            yt = None
            for c in range(NCH):
                if c % (XS // CHUNK) == 0:
                    xt = xp.tile([4, XS], F32R, tag="x")
                    nc.sync.dma_start(
                        out=xt[:], in_=x_d[:, c * CHUNK:c * CHUNK + XS])
                xo = (c % (XS // CHUNK)) * CHUNK
                for d, w0t in ((0, w0pt), (1, w0nt)):
                    z0 = zp.tile([128, CHUNK], F32, tag="z")
                    for q in range(SB):
                        nc.tensor.matmul(
                            out=z0[:, q * 512:(q + 1) * 512],
                            lhsT=w0t[:],
                            rhs=xt[:, xo + q * 512:xo + (q + 1) * 512]
                                ,
                            start=True, stop=True)
                    h0 = hp.tile([128, CHUNK], F32R, tag="h")
                    nc.scalar.activation(out=h0[:], in_=z0[:], func=AF.Silu,
                                         bias=b0t[:])
                    z1 = zp.tile([128, CHUNK], F32, tag="z")
                    for q in range(SB):
                        nc.tensor.matmul(
                            out=z1[:, q * 512:(q + 1) * 512],
                            lhsT=w1t[:],
                            rhs=h0[:, q * 512:(q + 1) * 512],
                            start=True, stop=True)
                    h1 = hp.tile([128, CHUNK], F32R, tag="h")
                    nc.scalar.activation(out=h1[:], in_=z1[:], func=AF.Silu,
                                         bias=b1t[:])
                    for q in range(SB):
                        s = c * (2 * SB) + d * SB + q
                        p = s % 128
                        if p == 0:
                            if yt is not None:
                                ysb = ysp.tile([128, 512], F32, tag="ysb")
                                nc.vector.tensor_copy(out=ysb[:], in_=yt[:])
                                nc.sync.dma_start(
                                    out=y_d[s - 128:s, :], in_=ysb[:])
                            yt = yp.tile([128, 512], F32, tag="y")
                        nc.tensor.matmul(
                            out=yt[:],
                            lhsT=w2et[:, p * 128:(p + 1) * 128],
                            rhs=h1[:, q * 512:(q + 1) * 512],
                            start=(p == 0), stop=(p == 127 or s == SPC - 1),
                            skip_group_check=True)
            last = SPC % 128 or 128
            ysb = ysp.tile([128, 512], F32, tag="ysb")
            nc.vector.tensor_copy(out=ysb[0:last, :], in_=yt[0:last, :])
            nc.sync.dma_start(out=y_d[SPC - last:SPC, :], in_=ysb[0:last, :])
    return nc


def prepare(v, r_ij, W0, b0, W1, b1, W2, b2, edge_index):
    """Host prep: returns (nc, in_maps, postprocess_fn)."""
    v = np.asarray(v, np.float32)
    r_ij = np.asarray(r_ij, np.float32)
    ei = np.asarray(edge_index)
    i_all = ei[0].astype(np.int64)
    j_all = ei[1].astype(np.int64)

    W0 = np.asarray(W0, np.float32)          # [128, 4]
    w0p = np.ascontiguousarray(W0.T)         # [4, 128] lhsT
    w0n = w0p.copy()
    w0n[1:4, :] *= -1.0                      # negate v-feature rows for dir j
    w2col = np.asarray(W2, np.float32).reshape(128)
    w2e = np.zeros((128, 128, 128), np.float32)
    w2e[:, np.arange(128), np.arange(128)] = w2col[:, None]
    wmap = {
        "w0p": w0p,
        "w0n": w0n,
        "w1lt": np.ascontiguousarray(np.asarray(W1, np.float32).T),
        "w2e": w2e.reshape(128, 128 * 128),
        "b0c": np.asarray(b0, np.float32).reshape(128, 1),
        "b1c": np.asarray(b1, np.float32).reshape(128, 1),
    }
    b2val = float(np.asarray(b2).reshape(()))

    vij_all = v[i_all] - v[j_all]                            # [E, 3]
    r_all = np.sqrt((r_ij * r_ij).sum(1)) * np.float32(1.0 / HSM)  # [E]
    x_all = np.empty((4, E_TOTAL), np.float32)
    x_all[0] = r_all
    x_all[1:4] = vij_all.T

    in_maps = []
    for c in range(N_CORES):
        x = np.zeros((4, K), np.float32)
        x[:, :Epc] = x_all[:, c * Epc:(c + 1) * Epc]
        m = {"x": x}
        m.update(wmap)
        in_maps.append(m)

    nc = _build_kernel()
    _split_multi_waits(nc)

    def post(results):
        S_i = np.zeros(N_NODES, np.float64)
        S_j = np.zeros(N_NODES, np.float64)
        c_i = np.bincount(i_all, minlength=N_NODES)
        c_j = np.bincount(j_all, minlength=N_NODES)
        for c in range(N_CORES):
            Y = np.asarray(results[c]["y"])          # [SPC, 512]
            Yr = Y.reshape(NCH, 2, SB * 512)         # s = (chunk, dir, sub)
            mi = Yr[:, 0, :].reshape(K)[:Epc]
            mj = Yr[:, 1, :].reshape(K)[:Epc]
            sl = slice(c * Epc, (c + 1) * Epc)
            np.add.at(S_i, i_all[sl], mi)
            np.add.at(S_j, j_all[sl], mj)
        S = (S_i / np.maximum(c_i, 1) + b2val * (c_i > 0)
             + S_j / np.maximum(c_j, 1) + b2val * (c_j > 0))
        return S[:, None].astype(np.float32)

    return nc, in_maps, post


def kernel(v, r_ij, W0, b0, W1, b1, W2, b2, edge_index):
    from concourse.bass_utils import run_bass_kernel_spmd
    nc, in_maps, post = prepare(v, r_ij, W0, b0, W1, b1, W2, b2,
                                edge_index)
    res = run_bass_kernel_spmd(nc, in_maps, core_ids=list(range(N_CORES)))
    return post(res.results)


# revision 13
# speedup vs baseline: 1.5837x; 1.5837x over previous
"""Trainium2 bass kernel for the GNN message-passing problem (v2).

kernel(**inputs) -> np.ndarray [100000, 1]

Strategy: edges sharded contiguously across 8 NeuronCores (200k/core).
Host packs per-edge features as x = [r, vx, vy, vz] in a [4, K]
feature-major layout (r = |r_ij|/H precomputed on host), so the device
needs no transposes: each teacher layer is a plain wide matmul in
float32r (fp32 precision at 1 cycle/row PE throughput).

Per 2048-edge chunk, per direction (dir j uses a stationary with the
v-columns negated, reusing the same rhs):
  z0 = W0 @ x      (4x 512-col matmuls -> 4-bank PSUM tile [128,2048])
  h0 = silu(z0+b0) (ONE 2048-wide activation -> SBUF; the activation
                    engine is the bottleneck and its ~450ns fixed cost
                    amortizes over width)
  z1 = W1 @ h0     (4x 512-col matmuls)
  h1 = silu(z1+b1)
  y  = w2 . h1     (1x 512-col matmul per 512-edge sub-block; the
                    stationary w2emb[t] = w2 embedded in column t of a
                    [128,128] zero matrix deposits the result into
                    partition t of an accumulating [128,512] PSUM
                    region - fp32r matmul PSUM writes must start at
                    partition 0, so rows are accumulated in, not
                    addressed. The region is bank 0 of the z1 tile,
                    free after the h1 silu: zbufs=2 x 4 banks uses all
                    8 PSUM banks for maximum silu width. Rows 0..7 are
                    drained per chunk by the otherwise-idle DVE.)
b2 and the index-dependent segment-sum/count/divide run on the host.
"""
import sys
sys.path.insert(0, "/opt/trn_rl_repo")
import numpy as np

N_NODES = 100000
E_TOTAL = 1600000
HSM = 3.0
N_CORES = 8
CHUNK = 2048            # edges per chunk (one 4-bank PSUM tile)
SB = CHUNK // 512       # 512-col sub-blocks per chunk-direction: 4
SPCH = 2 * SB           # sub-block-dirs per chunk: 8
Epc = E_TOTAL // N_CORES
XSC = 4                 # chunks per x super-tile DMA
NCH = -(-Epc // CHUNK)
NCH = -(-NCH // XSC) * XSC         # 100 (rounded up to XSC multiple)
K = NCH * CHUNK                    # 204800 padded edges per core
XS = XSC * CHUNK
SPC = NCH * SPCH                   # sub-block-dirs per core: 800


def _coalesce_and_split_waits(nc, max_waits=1):
    """Merge same-semaphore waits (keep max target), then move any
    remaining extra waits onto preceding single-wait InstNoOps (this
    walrus build rejects >1 sync-wait per instruction)."""
    import concourse.mybir as mybir
    n_split = 0
    for f in nc.m.functions:
        for blk in f.blocks:
            out = []
            for ins in blk.instructions:
                si = ins.sync_info
                if si is not None and si.on_wait and len(si.on_wait) > 1:
                    merged = {}
                    for w in si.on_wait:
                        key = (w.id, w.ant_name, str(w.sync_type),
                               str(w.wait_mode))
                        prev = merged.get(key)
                        if prev is None or w.wait_value > prev.wait_value:
                            merged[key] = w
                    waits = list(merged.values())
                    if len(waits) > max_waits:
                        for k, w in enumerate(waits[:-max_waits]):
                            nop = mybir.InstNoOp(name=f"{ins.name}-ws{k}")
                            nop.engine = ins.engine
                            nop.sync_info = mybir.SyncInfo(
                                on_wait=[w], on_update=[])
                            out.append(nop)
                            n_split += 1
                        waits = waits[-max_waits:]
                    si.on_wait = waits
                out.append(ins)
            blk.instructions = out
    return n_split


def _build_kernel():
    import concourse.bass as bass
    import concourse.mybir as mybir
    from concourse.tile import TileContext

    F32 = mybir.dt.float32
    F32R = mybir.dt.float32r
    AF = mybir.ActivationFunctionType

    nc = bass.Bass()
    x_d = nc.declare_dram_parameter("x", [4, K], F32R, isOutput=False)
    w0p_d = nc.declare_dram_parameter("w0p", [4, 128], F32R, isOutput=False)
    w0n_d = nc.declare_dram_parameter("w0n", [4, 128], F32R, isOutput=False)
    w1_d = nc.declare_dram_parameter("w1lt", [128, 128], F32R, isOutput=False)
    w2e_d = nc.declare_dram_parameter("w2e", [128, SPCH * 128], F32R,
                                      isOutput=False)
    b0_d = nc.declare_dram_parameter("b0c", [128, 1], F32, isOutput=False)
    b1_d = nc.declare_dram_parameter("b1c", [128, 1], F32, isOutput=False)
    y_d = nc.declare_dram_parameter("y", [SPC, 512], F32, isOutput=True)

    with TileContext(nc) as tc:
        with (
            tc.tile_pool(name="const", bufs=1) as cpool,
            tc.tile_pool(name="xp", bufs=2) as xp,
            tc.tile_pool(name="hp", bufs=6) as hp,
            tc.tile_pool(name="zp", bufs=2, space="PSUM") as zp,
            tc.tile_pool(name="ysp", bufs=2) as ysp,
        ):
            w0pt = cpool.tile([4, 128], F32R, tag="w0p")
            w0nt = cpool.tile([4, 128], F32R, tag="w0n")
            w1t = cpool.tile([128, 128], F32R, tag="w1")
            w2et = cpool.tile([128, SPCH * 128], F32R, tag="w2e")
            b0t = cpool.tile([128, 1], F32, tag="b0")
            b1t = cpool.tile([128, 1], F32, tag="b1")
            for tgt, src in ((w0pt, w0p_d), (w0nt, w0n_d), (w1t, w1_d),
                             (w2et, w2e_d), (b0t, b0_d), (b1t, b1_d)):
                nc.sync.dma_start(out=tgt[:], in_=src[:])

            xt = None
            for c in range(NCH):
                if c % XSC == 0:
                    xt = xp.tile([4, XS], F32R, tag="x")
                    nc.sync.dma_start(
                        out=xt[:], in_=x_d[:, c * CHUNK:c * CHUNK + XS])
                xo = (c % XSC) * CHUNK
                # Direction-interleaved schedule: while Act runs one silu,
                # PE runs the other direction's matmuls (disjoint banks).
                z0 = {}
                for d, w0t in ((0, w0pt), (1, w0nt)):
                    z0[d] = zp.tile([128, CHUNK], F32, tag="z", name=f"z0_{d}")
                    for q in range(SB):
                        nc.tensor.matmul(
                            out=z0[d][:, q * 512:(q + 1) * 512],
                            lhsT=w0t[:],
                            rhs=xt[:, xo + q * 512:xo + (q + 1) * 512],
                            start=True, stop=True)
                h0 = {}
                for d in (0, 1):
                    h0[d] = hp.tile([128, CHUNK], F32R, tag="h", name=f"h0_{d}")
                    nc.scalar.activation(out=h0[d][:], in_=z0[d][:],
                                         func=AF.Silu, bias=b0t[:])
                z1 = {}
                for d in (0, 1):
                    z1[d] = zp.tile([128, CHUNK], F32, tag="z", name=f"z1_{d}")
                    for q in range(SB):
                        nc.tensor.matmul(
                            out=z1[d][:, q * 512:(q + 1) * 512],
                            lhsT=w1t[:],
                            rhs=h0[d][:, q * 512:(q + 1) * 512],
                            start=True, stop=True)
                h1 = {}
                for d in (0, 1):
                    h1[d] = hp.tile([128, CHUNK], F32R, tag="h", name=f"h1_{d}")
                    nc.scalar.activation(out=h1[d][:], in_=z1[d][:],
                                         func=AF.Silu, bias=b1t[:])
                # y accumulation region: bank 0 of z1[1], free after its
                # silu; drained by DVE before the next chunk reuses it.
                for dd in (0, 1):
                    for q in range(SB):
                        t = dd * SB + q
                        nc.tensor.matmul(
                            out=z1[1][:, 0:512],
                            lhsT=w2et[:, t * 128:(t + 1) * 128],
                            rhs=h1[dd][:, q * 512:(q + 1) * 512],
                            start=(t == 0), stop=(t == SPCH - 1),
                            skip_group_check=True)
                ysb = ysp.tile([SPCH, 512], F32, tag="ysb")
                nc.vector.tensor_copy(out=ysb[:], in_=z1[1][0:SPCH, 0:512])
                nc.sync.dma_start(
                    out=y_d[c * SPCH:(c + 1) * SPCH, :], in_=ysb[:])
    return nc


def prepare(v, r_ij, W0, b0, W1, b1, W2, b2, edge_index):
    """Host prep: returns (nc, in_maps, postprocess_fn)."""
    v = np.asarray(v, np.float32)
    r_ij = np.asarray(r_ij, np.float32)
    ei = np.asarray(edge_index)
    i_all = ei[0].astype(np.int64)
    j_all = ei[1].astype(np.int64)

    W0 = np.asarray(W0, np.float32)          # [128, 4]
    w0p = np.ascontiguousarray(W0.T)         # [4, 128] lhsT
    w0n = w0p.copy()
    w0n[1:4, :] *= -1.0                      # negate v-feature rows for dir j
    w2col = np.asarray(W2, np.float32).reshape(128)
    w2e = np.zeros((128, SPCH, 128), np.float32)
    w2e[:, np.arange(SPCH), np.arange(SPCH)] = w2col[:, None]
    wmap = {
        "w0p": w0p,
        "w0n": w0n,
        "w1lt": np.ascontiguousarray(np.asarray(W1, np.float32).T),
        "w2e": w2e.reshape(128, SPCH * 128),
        "b0c": np.asarray(b0, np.float32).reshape(128, 1),
        "b1c": np.asarray(b1, np.float32).reshape(128, 1),
    }
    b2val = float(np.asarray(b2).reshape(()))

    vij_all = v[i_all] - v[j_all]                            # [E, 3]
    r_all = np.sqrt((r_ij * r_ij).sum(1)) * np.float32(1.0 / HSM)  # [E]
    x_all = np.empty((4, E_TOTAL), np.float32)
    x_all[0] = r_all
    x_all[1:4] = vij_all.T

    in_maps = []
    for c in range(N_CORES):
        x = np.zeros((4, K), np.float32)
        x[:, :Epc] = x_all[:, c * Epc:(c + 1) * Epc]
        m = {"x": x}
        m.update(wmap)
        in_maps.append(m)

    nc = _build_kernel()
    _coalesce_and_split_waits(nc)

    def post(results):
        S_i = np.zeros(N_NODES, np.float64)
        S_j = np.zeros(N_NODES, np.float64)
        c_i = np.bincount(i_all, minlength=N_NODES)
        c_j = np.bincount(j_all, minlength=N_NODES)
        for c in range(N_CORES):
            Y = np.asarray(results[c]["y"])          # [SPC, 512]
            Yr = Y.reshape(NCH, 2, SB * 512)         # s = (chunk, dir, sub)
            mi = Yr[:, 0, :].reshape(K)[:Epc]
            mj = Yr[:, 1, :].reshape(K)[:Epc]
            sl = slice(c * Epc, (c + 1) * Epc)
            np.add.at(S_i, i_all[sl], mi)
            np.add.at(S_j, j_all[sl], mj)
        S = (S_i / np.maximum(c_i, 1) + b2val * (c_i > 0)
             + S_j / np.maximum(c_j, 1) + b2val * (c_j > 0))
        return S[:, None].astype(np.float32)

    return nc, in_maps, post


def kernel(v, r_ij, W0, b0, W1, b1, W2, b2, edge_index):
    from concourse.bass_utils import run_bass_kernel_spmd
    nc, in_maps, post = prepare(v, r_ij, W0, b0, W1, b1, W2, b2,
                                edge_index)
    res = run_bass_kernel_spmd(nc, in_maps, core_ids=list(range(N_CORES)))
    return post(res.results)
